# revision 1
# baseline (speedup 1.0000x reference)
"""Trainium2 SPMD kernel for NativeSparseAttention (B=2,S=1024,D=1024,H=16,HD=64).

Sharding: tensor-parallel over heads. 8 cores x 2 heads (128 cols of D) each.
Every core computes:
  - its head-shard of q/k/v (bf16), all three branch outputs for its heads,
  - a replicated fp32 "importance" chain (exact top-2 block selection),
  - a partial out@Wo (rows of Wo owned by its heads).
Host sums the 8 partial [2048,1024] outputs (the only cross-core reduction).

Branch 2 (top-k block selection) is computed densely against the 512
selectable tokens per source batch; the data-dependent selection enters as a
one-hot multiplicative mask on exp(scores) built with iota==top_idx compares.
No gathers, no collectives.

Layouts are feature-major ("transposed", [feature, token]) so matmuls chain
without transposing activations; only V and the one-hot masks are transposed
on the PE. Each branch's attention-value matmul carries an extra ones-column
so the softmax denominator comes out as row 64 of the same PSUM tile ("aug"
trick); the denominator row is moved to partition 0 with a tiny SBUF->SBUF
DMA (engines cannot shift partitions).
"""

import math
from contextlib import ExitStack

import numpy as np
import ml_dtypes

B, S, D = 2, 1024, 1024
H, HD = 16, 64
CB, SB, J, W = 16, 8, 2, 256
Nc = S // CB  # 64 compressed blocks
T = B * S  # 2048 tokens total
NCORES = 8
HPC = H // NCORES  # 2 heads per core
C = HPC * HD  # 128 feature cols per core
INV = 1.0 / math.sqrt(HD)

bf16 = ml_dtypes.bfloat16

_PROG = None  # cached nc


def _build_program():
    import concourse.bass as bass
    import concourse.bacc as bacc
    import concourse.mybir as mybir
    import concourse.tile as tile
    from concourse.masks import make_identity

    dt = mybir.dt
    Alu = mybir.AluOpType
    Act = mybir.ActivationFunctionType

    nc = bacc.Bacc("TRN2", target_bir_lowering=False, debug=False,
                   num_devices=NCORES)

    # ---- DRAM parameters (per-core data supplied by the host) ----
    xT32 = nc.declare_dram_parameter("xT32", [D, T], dt.float32, isOutput=False)
    xT16 = nc.declare_dram_parameter("xT16", [D, T], dt.bfloat16, isOutput=False)
    wkwq = nc.declare_dram_parameter("wkwq", [D, D], dt.float32, isOutput=False)
    wq16 = nc.declare_dram_parameter("wq16", [D, C], dt.bfloat16, isOutput=False)
    wk16 = nc.declare_dram_parameter("wk16", [D, C], dt.bfloat16, isOutput=False)
    wv16 = nc.declare_dram_parameter("wv16", [D, C], dt.bfloat16, isOutput=False)
    wo16 = nc.declare_dram_parameter("wo16", [C, D], dt.bfloat16, isOutput=False)
    wg16 = nc.declare_dram_parameter("wg16", [D, 3], dt.bfloat16, isOutput=False)
    wpeT = nc.declare_dram_parameter("wpeT", [C, CB], dt.float32, isOutput=False)
    wkcB = nc.declare_dram_parameter("wkcB", [128, CB], dt.float32, isOutput=False)
    wvcB = nc.declare_dram_parameter("wvcB", [128, CB], dt.float32, isOutput=False)
    c1kcol = nc.declare_dram_parameter("c1kcol", [C, 1], dt.float32, isOutput=False)
    c1vB = nc.declare_dram_parameter("c1vB", [128, C], dt.float32, isOutput=False)
    bgcol = nc.declare_dram_parameter("bgcol", [3, 1], dt.float32, isOutput=False)
    bo32 = nc.declare_dram_parameter("bo32", [1, D], dt.float32, isOutput=False)
    iota64 = nc.declare_dram_parameter("iota64", [128, Nc], dt.float32, isOutput=False)
    mask0 = nc.declare_dram_parameter("mask0", [128, 128], dt.float32, isOutput=False)
    mask2 = nc.declare_dram_parameter("mask2", [128, 128], dt.float32, isOutput=False)
    onesb = nc.declare_dram_parameter("onesb", [1, 128], dt.bfloat16, isOutput=False)
    ones32 = nc.declare_dram_parameter("ones32", [1, 128], dt.float32, isOutput=False)

    out_part = nc.declare_dram_parameter("out_part", [T, D], dt.float32, isOutput=True)

    with tile.TileContext(nc) as tc, ExitStack() as ctx:
        sync = nc.sync

        def pool(name, bufs=1, space="SBUF"):
            return ctx.enter_context(tc.tile_pool(name=name, bufs=bufs, space=space))

        def arr(p, n, shape, dtype, tag):
            return [p.tile(shape, dtype, tag=f"{tag}{i}", name=f"{tag}{i}")
                    for i in range(n)]

        # pools for phase A only (x fp32/bf16, WkWq^T, M, top-k scratch);
        # released after phase 3 so branch-phase pools can reuse the SBUF
        ctxA = ExitStack()

        def poolA(name, bufs=1):
            return ctxA.enter_context(tc.tile_pool(name=name, bufs=bufs))

        # ---------- constants ----------
        cpool = pool("consts")
        c_wkc = cpool.tile([128, CB], dt.float32, tag="wkc", name="wkc")
        c_wvc = cpool.tile([128, CB], dt.float32, tag="wvc", name="wvc")
        c_wpeT = cpool.tile([C, CB], dt.float32, tag="wpeT", name="wpeT")
        c_c1k = cpool.tile([C, 1], dt.float32, tag="c1k", name="c1k")
        c_c1v = cpool.tile([128, C], dt.float32, tag="c1v", name="c1v")
        c_bg = cpool.tile([3, 1], dt.float32, tag="bg", name="bg")

        c_iota = cpool.tile([128, Nc], dt.float32, tag="iota", name="iota")
        c_m0 = cpool.tile([128, 128], dt.float32, tag="m0", name="m0")
        c_m2 = cpool.tile([128, 128], dt.float32, tag="m2", name="m2")
        c_1b = cpool.tile([1, 128], dt.bfloat16, tag="onesb", name="onesb")
        c_132 = cpool.tile([1, 128], dt.float32, tag="ones32", name="ones32")
        c_ident = cpool.tile([128, 128], dt.bfloat16, tag="ident", name="ident")
        c_1b64 = cpool.tile([65, 128], dt.bfloat16, tag="ones65", name="ones65")
        nc.gpsimd.memset(c_1b64[:], 1.0)
        for t_, src in [
            (c_wkc, wkcB), (c_wvc, wvcB), (c_wpeT, wpeT), (c_c1k, c1kcol),
            (c_c1v, c1vB), (c_bg, bgcol), (c_iota, iota64),
            (c_m0, mask0), (c_m2, mask2), (c_1b, onesb), (c_132, ones32),
        ]:
            sync.dma_start(t_[:], src[:])
        make_identity(nc, c_ident[:])

        wpool = pool("wpool")
        wq = arr(wpool, 8, [128, C], dt.bfloat16, "wq_")
        wk = arr(wpool, 8, [128, C], dt.bfloat16, "wk_")
        wv = arr(wpool, 8, [128, C], dt.bfloat16, "wv_")
        wg = arr(wpool, 8, [128, 3], dt.bfloat16, "wg_")
        for i in range(8):
            sl = slice(i * 128, (i + 1) * 128)
            sync.dma_start(wq[i][:], wq16[sl, :])
            sync.dma_start(wk[i][:], wk16[sl, :])
            sync.dma_start(wv[i][:], wv16[sl, :])
            sync.dma_start(wg[i][:], wg16[sl, :])

        # PSUM pools: psA (2 banks) + psS (4 banks) + psb2 (2 banks) = 8
        psA = pool("psA", bufs=2, space="PSUM")
        psS = pool("psS", bufs=4, space="PSUM")
        psb2 = pool("psb2", bufs=1, space="PSUM")

        def pa():
            return psA.tile([128, 512], dt.float32, tag="pa", name="pa")

        def ps(p_=128, f=512):
            t_ = psS.tile([128, 512], dt.float32, tag="ps", name="ps")
            return t_[0:p_, 0:f]

        def psT(p_=128, f=512):
            t_ = psS.tile([128, 512], dt.bfloat16, tag="ps", name="psT")
            return t_[0:p_, 0:f]

        # ---------- persistent pools (created before phase-A scope: LIFO) ----------
        cxp = pool("cxp")
        cxkT = arr(cxp, 8, [128, 128], dt.float32, "cxk_")
        cxvT = arr(cxp, 8, [128, 128], dt.float32, "cxv_")
        cxkT16 = arr(cxp, 8, [128, 128], dt.bfloat16, "cxk16_")
        cxvT16 = arr(cxp, 8, [128, 128], dt.bfloat16, "cxv16_")
        actp = pool("actp")
        qT = actp.tile([C, T], dt.bfloat16, tag="qT", name="qT")
        kT = actp.tile([C, T], dt.bfloat16, tag="kT", name="kT")
        vT = actp.tile([C, T], dt.bfloat16, tag="vT", name="vT")
        growb = actp.tile([3, T], dt.bfloat16, tag="growb", name="growb")
        otp = pool("otp")
        # OT[src][rank]: [64, 1024] bf16; masks output-batch `rank` tokens
        OT = [[otp.tile([Nc, S], dt.bfloat16, tag=f"OT{j}{r}", name=f"OT{j}{r}")
               for r in range(2)] for j in range(2)]

        # ---------- big inputs (phase-A scoped pools) ----------
        xt32 = arr(poolA("xt32p"), 8, [128, T], dt.float32, "xt32_")
        wkwqt = arr(poolA("wkwqp"), 8, [128, D], dt.float32, "wkwq_")
        xt16 = arr(poolA("xt16p"), 8, [128, T], dt.bfloat16, "xt16_")
        for i in range(8):
            sl = slice(i * 128, (i + 1) * 128)
            sync.dma_start(xt32[i][:], xT32[sl, :])
            sync.dma_start(wkwqt[i][:], wkwq[sl, :])
            sync.dma_start(xt16[i][:], xT16[sl, :])

        # =========================================================
        # Phase 1: cx weighted sums (DVE): cxkT/cxvT [d,128bn] f32
        # cx[d, b*64+n] = sum_t w[t] * xT[d, b*1024 + n*16 + t]
        # =========================================================
        for i in range(8):
            xv = xt32[i][:].rearrange("p (b n t) -> p (b n) t", b=B, t=CB)
            for (acc, wt, acc16) in ((cxkT[i], c_wkc, cxkT16[i]),
                                     (cxvT[i], c_wvc, cxvT16[i])):
                for t in range(CB):
                    if t == 0:
                        nc.vector.tensor_scalar_mul(
                            acc[:], xv[:, :, t], wt[:, 0:1])
                    else:
                        nc.vector.scalar_tensor_tensor(
                            out=acc[:], in0=xv[:, :, t], scalar=wt[:, t:t + 1],
                            in1=acc[:], op0=Alu.mult, op1=Alu.add)
                nc.vector.tensor_copy(acc16[:], acc[:])

        # =========================================================
        # Phase 2: projections qT/kT/vT [C, T] bf16 (+pe for k,v), gates
        # =========================================================
        for ch in range(4):
            csl = slice(ch * 512, (ch + 1) * 512)
            for which, wtile, dest in (("q", wq, qT), ("k", wk, kT), ("v", wv, vT)):
                pp = pa()
                for i in range(8):
                    nc.tensor.matmul(pp[:], wtile[i][:], xt16[i][:, csl],
                                     start=(i == 0), stop=(i == 7))
                if which == "q":
                    nc.scalar.copy(dest[:, csl], pp[:])
                else:
                    # add tiled positional rows: out = psum + wpeT (tok%16)
                    dv = dest[:, csl].rearrange("p (r t) -> p r t", t=CB)
                    pv = pp[:].rearrange("p (r t) -> p r t", t=CB)
                    peb = c_wpeT[:][:, None, :].to_broadcast((C, 512 // CB, CB))
                    nc.vector.tensor_tensor(out=dv, in0=pv, in1=peb, op=Alu.add)
            pg = ps(3, 512)
            for i in range(8):
                nc.tensor.matmul(pg[:], wg[i][:], xt16[i][:, csl],
                                 start=(i == 0), stop=(i == 7))
            nc.scalar.activation(growb[:, csl], pg[:], Act.Sigmoid, bias=c_bg[:])

        # =========================================================
        # Phase 3: M[e,128bn] f32 = sum_d wkwq[d,e]*cxkT[d,bn]; imp; top2; O
        # =========================================================
        Mt = arr(poolA("mp"), 8, [128, 128], dt.float32, "M_")
        for e in range(8):
            pm = ps(128, 128)
            for i in range(8):
                nc.tensor.matmul(pm[:], wkwqt[i][:, e * 128:(e + 1) * 128],
                                 cxkT[i][:], start=(i == 0), stop=(i == 7))
            nc.scalar.copy(Mt[e][:], pm[:])

        idxp = poolA("idxp", bufs=3)
        for b in range(B):
            for st in range(8):
                ssl = slice(b * S + st * 128, b * S + (st + 1) * 128)
                pi = ps(128, Nc)
                for e in range(8):
                    nc.tensor.matmul(pi[:], xt32[e][:, ssl],
                                     Mt[e][:, b * Nc:(b + 1) * Nc],
                                     start=(e == 0), stop=(e == 7))
                impt = idxp.tile([128, Nc], dt.float32, tag="impt", name="impt")
                nc.vector.tensor_copy(impt[:], pi[:])
                mx8 = idxp.tile([128, 8], dt.float32, tag="mx8", name="mx8")
                ix8 = idxp.tile([128, 8], dt.uint32, tag="ix8", name="ix8")
                nc.vector.max(out=mx8[:], in_=impt[:])
                nc.vector.max_index(out=ix8[:], in_max=mx8[:], in_values=impt[:])
                ixf = idxp.tile([128, 2], dt.float32, tag="ixf", name="ixf")
                nc.vector.tensor_copy(ixf[:], ix8[:, 0:2])
                for r in range(2):
                    oh = idxp.tile([128, Nc], dt.bfloat16, tag="oh", name="oh")
                    nc.vector.tensor_scalar(
                        out=oh[:], in0=c_iota[:], scalar1=ixf[:, r:r + 1],
                        scalar2=None, op0=Alu.is_equal)
                    pt = psT(Nc, 128)
                    nc.tensor.transpose(pt[:], oh[:], c_ident[:])
                    nc.vector.tensor_copy(OT[b][r][:, st * 128:(st + 1) * 128],
                                          pt[:])

        ctxA.close()  # release xT32/wkwq/xT16/M/idx SBUF for later phases

        latep = pool("latep")
        wo_h = [latep.tile([HD, D], dt.bfloat16, tag=f"wo{h}", name=f"wo{h}")
                for h in range(2)]
        sync.dma_start(wo_h[0][:], wo16[0:HD, :])
        sync.dma_start(wo_h[1][:], wo16[HD:C, :])
        c_bo = latep.tile([1, D], dt.float32, tag="bo", name="bo")
        sync.dma_start(c_bo[:], bo32[:])

        # =========================================================
        # Phase 4: ckT shard (+c1k), cv_aug (+c1v, ones col), v_tok, Vblk
        # =========================================================
        ckp = pool("ckp")
        ckTs = ckp.tile([C, 128], dt.bfloat16, tag="ckTs", name="ckTs")    # [c, b*64+n]
        # cva[b]: [n, h*65+c] bf16 with ones col at 64/129 (per batch, base 0)
        cva = [ckp.tile([Nc, 130], dt.bfloat16, tag=f"cva{b}", name=f"cva{b}")
               for b in range(B)]
        pc = ps(C, 128)
        for i in range(8):
            nc.tensor.matmul(pc[:], wk[i][:], cxkT16[i][:],
                             start=(i == 0), stop=(i == 7))
        nc.vector.tensor_scalar_add(ckTs[:], pc[:], c_c1k[:])
        for b in range(B):
            pc = ps(Nc, C)
            for i in range(8):
                nc.tensor.matmul(pc[:], cxvT16[i][:, b * Nc:(b + 1) * Nc],
                                 wv[i][:], start=(i == 0), stop=(i == 7))
            cva_v = cva[b][:].rearrange("p (h c) -> p h c", h=2)
            nc.vector.tensor_tensor(
                out=cva_v[:, :, 0:64],
                in0=pc[:].rearrange("p (h c) -> p h c", h=2),
                in1=c_c1v[0:Nc, :].rearrange("p (h c) -> p h c", h=2),
                op=Alu.add)
            nc.gpsimd.memset(cva_v[:, :, 64:65], 1.0)

        # v_tok: [tok, 130] bf16 x16 tiles (natural order, for branch 3)
        v_tok = arr(pool("vtp"), 16, [128, 130], dt.bfloat16, "vtok_")
        for tt in range(16):
            pv = psT(128, 128)
            nc.tensor.transpose(pv[:], vT[:, tt * 128:(tt + 1) * 128], c_ident[:])
            dv = v_tok[tt][:].rearrange("p (h c) -> p h c", h=2)
            nc.vector.tensor_copy(
                dv[:, :, 0:64], pv[:].rearrange("p (h c) -> p h c", h=2))
            nc.gpsimd.memset(dv[:, :, 64:65], 1.0)

        # Vblk[(src,t)]: [64 n, 130] bf16 — selectable tokens src*1024+n*16+t
        vbp = pool("vbp")
        Vblk = [[vbp.tile([Nc, 130], dt.bfloat16, tag=f"vb{src}{t}", name=f"vb{src}{t}")
                 for t in range(SB)] for src in range(2)]
        vTv = vT[:].rearrange("p (b n t) -> p b t n", b=B, t=CB)
        for src in range(2):
            for t in range(SB):
                pv = psT(Nc, 128)
                nc.tensor.transpose(pv[:], vTv[:, src, t, :], c_ident[:])
                dv = Vblk[src][t][:].rearrange("p (h c) -> p h c", h=2)
                nc.vector.tensor_copy(
                    dv[:, :, 0:64], pv[:].rearrange("p (h c) -> p h c", h=2))
                nc.gpsimd.memset(dv[:, :, 64:65], 1.0)

        # =========================================================
        # Phases 5-7: branches. ofull[k][b][h]: [65, S] bf16
        #   rows 0-63 = numerator values, row 64 = softmax denominator.
        # =========================================================
        brp = pool("brp")
        ofull = [[[brp.tile([HD + 1, S], dt.bfloat16, tag=f"o{k}_{b}{h}", name=f"o{k}_{b}{h}")
                   for h in range(2)] for b in range(2)] for k in range(3)]
        expp = pool("expp", bufs=4)

        # ---- branch 1: compressed attention ----
        for b in range(B):
            for h in range(HPC):
                hsl = slice(h * HD, (h + 1) * HD)
                asl = slice(h * 65, (h + 1) * 65)
                p1 = expp.tile([Nc, S], dt.bfloat16, tag="p1t", name="p1t")
                for ch in range(2):
                    csl = slice(ch * 512, (ch + 1) * 512)
                    tsl = slice(b * S + ch * 512, b * S + (ch + 1) * 512)
                    pp = ps(Nc, 512)
                    nc.tensor.matmul(pp[:], ckTs[hsl, b * Nc:(b + 1) * Nc],
                                     qT[hsl, tsl], start=True, stop=True)
                    nc.scalar.activation(p1[:, csl], pp[:], Act.Exp, scale=INV)
                for ch in range(2):
                    csl = slice(ch * 512, (ch + 1) * 512)
                    po = ps(HD + 1, 512)
                    nc.tensor.matmul(po[:], cva[b][:, asl],
                                     p1[:, csl], start=True, stop=True)
                    nc.scalar.copy(ofull[0][b][h][:, csl], po[:])

        # ---- branch 2: selected-block attention (dense + one-hot mask) ----
        kTv = kT[:].rearrange("p (b n t) -> p b t n", b=B, t=CB)
        for b in range(B):
            for h in range(HPC):
                hsl = slice(h * HD, (h + 1) * HD)
                asl = slice(h * 65, (h + 1) * 65)
                po2 = [psb2.tile([HD + 1, 512], dt.float32, tag=f"acc{ch}", name=f"acc{ch}")
                       for ch in range(2)]
                nslab = 2 * SB
                si = 0
                for src in range(2):
                    for t in range(SB):
                        p2 = expp.tile([Nc, S], dt.bfloat16, tag="p2t", name="p2t")
                        for ch in range(2):
                            csl = slice(ch * 512, (ch + 1) * 512)
                            tsl = slice(b * S + ch * 512, b * S + (ch + 1) * 512)
                            pp = ps(Nc, 512)
                            nc.tensor.matmul(pp[:], kTv[hsl, src, t, :],
                                             qT[hsl, tsl], start=True, stop=True)
                            nc.scalar.activation(p2[:, csl], pp[:], Act.Exp,
                                                 scale=INV)
                        # mask with one-hot of the selected block (slot j=src)
                        nc.vector.tensor_tensor(out=p2[:], in0=p2[:],
                                                in1=OT[src][b][:], op=Alu.mult)
                        for ch in range(2):
                            csl = slice(ch * 512, (ch + 1) * 512)
                            nc.tensor.matmul(
                                po2[ch][:], Vblk[src][t][:, asl], p2[:, csl],
                                start=(si == 0), stop=(si == nslab - 1))
                        si += 1
                for ch in range(2):
                    csl = slice(ch * 512, (ch + 1) * 512)
                    nc.scalar.copy(ofull[1][b][h][:, csl], po2[ch][:])

        # ---- branch 3: causal sliding window ----
        for b in range(B):
            for h in range(HPC):
                hsl = slice(h * HD, (h + 1) * HD)
                asl = slice(h * 65, (h + 1) * 65)
                for st in range(8):
                    ssl = slice(b * S + st * 128, b * S + (st + 1) * 128)
                    deltas = [d for d in (2, 1, 0) if st - d >= 0]
                    po = ps(HD + 1, 128)
                    for di, d in enumerate(deltas):
                        kt = st - d
                        ktsl = slice(b * S + kt * 128, b * S + (kt + 1) * 128)
                        pp = ps(128, 128)
                        nc.tensor.matmul(pp[:], kT[hsl, ktsl], qT[hsl, ssl],
                                         start=True, stop=True)
                        pext = expp.tile([128, 128], dt.bfloat16, tag="p3t", name="p3t")
                        if d == 1:
                            nc.scalar.activation(pext[:], pp[:], Act.Exp,
                                                 scale=INV)
                        else:
                            msk = c_m0 if d == 0 else c_m2
                            sm = expp.tile([128, 128], dt.float32, tag="b3m", name="b3m")
                            nc.vector.scalar_tensor_tensor(
                                out=sm[:], in0=pp[:], scalar=INV, in1=msk[:],
                                op0=Alu.mult, op1=Alu.add)
                            nc.scalar.activation(pext[:], sm[:], Act.Exp)
                        nc.tensor.matmul(
                            po[:], v_tok[b * 8 + kt][:, asl], pext[:],
                            start=(di == 0), stop=(di == len(deltas) - 1))
                    osl = slice(st * 128, (st + 1) * 128)
                    nc.scalar.copy(ofull[2][b][h][:, osl], po[:])

        # gate rows to partition 0 (SBUF->SBUF DMA; matmul lhsT/rhs bases must match)
        g_row = [brp.tile([1, T], dt.bfloat16, tag=f"grow{k}", name=f"grow{k}")
                 for k in range(3)]
        for k in range(3):
            sync.dma_start(g_row[k][:], growb[k:k + 1, :])

        # =========================================================
        # Phase 8: gating. gatedT_h [64, T] bf16 = sum_k g_k*recip(den_k)*o_k
        # =========================================================
        gp = pool("gp", bufs=2)
        gatedT = [gp.tile([HD, T], dt.bfloat16, tag=f"gated{h}", name=f"gated{h}") for h in range(2)]
        for b in range(B):
            for h in range(HPC):
                for k in range(3):
                    rg = gp.tile([HD, S], dt.float32, tag="rg", name="rg")
                    for ch in range(2):
                        csl = slice(ch * 512, (ch + 1) * 512)
                        tsl = slice(b * S + ch * 512, b * S + (ch + 1) * 512)
                        pd = ps(HD, 512)
                        nc.tensor.matmul(pd[:], c_1b64[64:65, 0:HD],
                                         ofull[k][b][h][HD:HD + 1, csl],
                                         start=True, stop=True)
                        pg2 = ps(HD, 512)
                        nc.tensor.matmul(pg2[:], c_1b[:, 0:HD],
                                         g_row[k][:, tsl],
                                         start=True, stop=True)
                        nc.vector.reciprocal(rg[:, csl], pd[:])
                        nc.vector.tensor_tensor(out=rg[:, csl], in0=rg[:, csl],
                                                in1=pg2[:], op=Alu.mult)
                    dst = gatedT[h][:, b * S:(b + 1) * S]
                    ob = ofull[k][b][h][0:HD, :]
                    if k == 0:
                        nc.vector.tensor_tensor(out=dst, in0=ob, in1=rg[:],
                                                op=Alu.mult)
                    else:
                        tmp = gp.tile([HD, S], dt.bfloat16, tag="gtmp", name="gtmp")
                        nc.vector.tensor_tensor(out=tmp[:], in0=ob, in1=rg[:],
                                                op=Alu.mult)
                        nc.vector.tensor_tensor(out=dst, in0=dst, in1=tmp[:],
                                                op=Alu.add)

        # =========================================================
        # Phase 9: out_part[s,:] = gatedT^T @ wo16 (+bo via per-core data)
        # =========================================================
        fop = pool("fop", bufs=3)
        for st in range(16):
            ssl = slice(st * 128, (st + 1) * 128)
            ot = fop.tile([128, D], dt.float32, tag="fo", name="fo")
            for ch in range(2):
                csl = slice(ch * 512, (ch + 1) * 512)
                pf = pa()
                nc.tensor.matmul(pf[:], gatedT[0][:, ssl], wo_h[0][:, csl],
                                 start=True, stop=False)
                nc.tensor.matmul(pf[:], gatedT[1][:, ssl], wo_h[1][:, csl],
                                 start=False, stop=False)
                nc.tensor.matmul(pf[:], c_132[:], c_bo[:, csl],
                                 start=False, stop=True, skip_group_check=True)
                nc.scalar.copy(ot[:, csl], pf[:])
            sync.dma_start(out_part[ssl, :], ot[:])

    nc.compile()
    return nc


def _prep_inputs(inputs):
    """Build the 8 per-core input maps from the full problem inputs."""
    x = np.ascontiguousarray(np.asarray(inputs["x"], dtype=np.float32))
    Wq = np.asarray(inputs["Wq"], dtype=np.float32)
    Wk = np.asarray(inputs["Wk"], dtype=np.float32)
    Wv = np.asarray(inputs["Wv"], dtype=np.float32)
    Wo = np.asarray(inputs["Wo"], dtype=np.float32)
    bo = np.asarray(inputs["bo"], dtype=np.float32)
    Wg = np.asarray(inputs["Wg"], dtype=np.float32)
    bg = np.asarray(inputs["bg"], dtype=np.float32)
    wkc = np.asarray(inputs["wkc"], dtype=np.float32)
    wvc = np.asarray(inputs["wvc"], dtype=np.float32)
    wpe = np.asarray(inputs["wpe"], dtype=np.float32)

    xT = np.ascontiguousarray(x.reshape(T, D).T)          # [D, T] f32
    xT16 = xT.astype(bf16)
    wkwq = np.ascontiguousarray(Wk @ Wq.T)                # [D, D] f32
    c1k = wkc @ wpe                                       # [D]
    c1v = wvc @ wpe
    iota = np.tile(np.arange(Nc, dtype=np.float32), (128, 1))
    ii = np.arange(128)[:, None]
    jj = np.arange(128)[None, :]
    m0 = np.where(jj >= ii, 0.0, -1e4).astype(np.float32)  # delta=0 keep col>=p
    m2 = np.where(jj <= ii, 0.0, -1e4).astype(np.float32)  # delta=2 keep col<=p
    wkcB = np.ascontiguousarray(np.tile(wkc, (128, 1)).astype(np.float32))
    wvcB = np.ascontiguousarray(np.tile(wvc, (128, 1)).astype(np.float32))
    bgcol = np.ascontiguousarray(bg.reshape(3, 1).astype(np.float32))
    onesb = np.ones((1, 128), bf16)
    ones32 = np.ones((1, 128), np.float32)

    in_maps = []
    for i in range(NCORES):
        csl = slice(i * C, (i + 1) * C)
        m = {
            "xT32": xT,
            "xT16": xT16,
            "wkwq": wkwq,
            "wq16": np.ascontiguousarray(Wq[:, csl]).astype(bf16),
            "wk16": np.ascontiguousarray(Wk[:, csl]).astype(bf16),
            "wv16": np.ascontiguousarray(Wv[:, csl]).astype(bf16),
            "wo16": np.ascontiguousarray(Wo[csl, :]).astype(bf16),
            "wg16": np.ascontiguousarray(Wg).astype(bf16),
            "wpeT": np.ascontiguousarray(wpe.T[csl, :]).astype(np.float32),
            "wkcB": wkcB,
            "wvcB": wvcB,
            "c1kcol": np.ascontiguousarray(c1k[csl].reshape(C, 1)),
            "c1vB": np.ascontiguousarray(np.tile(c1v[csl], (128, 1))
                                         .astype(np.float32)),
            "bgcol": bgcol,
            "bo32": np.ascontiguousarray(
                (bo if i == 0 else np.zeros_like(bo)).reshape(1, D)),
            "iota64": iota,
            "mask0": m0,
            "mask2": m2,
            "onesb": onesb,
            "ones32": ones32,
        }
        in_maps.append(m)
    return in_maps


_LAST_RESULTS = None


def kernel(**inputs) -> np.ndarray:
    global _PROG, _LAST_RESULTS
    import os
    from concourse.bass_utils import run_bass_kernel_spmd

    if _PROG is None:
        _PROG = _build_program()
    nc = _PROG

    in_maps = _prep_inputs(inputs)
    trace = bool(int(os.environ.get("KERNEL_TRACE", "0")))
    res = run_bass_kernel_spmd(nc, in_maps, core_ids=list(range(NCORES)),
                               trace=trace)
    _LAST_RESULTS = res
    total = np.zeros((T, D), np.float32)
    for i in range(NCORES):
        total += res.results[i]["out_part"]
    return total.reshape(B, S, D)



# revision 9
# speedup vs baseline: 1.6548x; 1.6548x over previous
"""Trainium2 SPMD kernel for NativeSparseAttention (B=2,S=1024,D=1024,H=16,HD=64).

Sharding: tensor-parallel over heads. 8 cores x 2 heads (128 cols of D) each.
Every core computes:
  - its head-shard of q/k/v (bf16), all three branch outputs for its heads,
  - a replicated fp32 "importance" chain (exact top-2 block selection),
  - a partial out@Wo (rows of Wo owned by its heads).
Host sums the 8 partial [2048,1024] outputs and adds bo.

Branch 2 (top-k block selection) is computed densely against the 1024
selectable tokens (both source batches stacked into 128-row key tiles); the
data-dependent selection enters as a one-hot multiplicative mask on
exp(scores) built with iota==top_idx compares. No gathers, no collectives.

Layouts are feature-major ("transposed", [feature, token]) so matmuls chain
without transposing activations; only V and the one-hot masks are transposed
on the PE. Each branch's attention-value matmul carries an extra ones-column
so the softmax denominator comes out as row 64 of the same PSUM tile ("aug"
trick). Gating transposes the denominators to token-partitions, takes cheap
reciprocals there, and broadcasts the per-token gate/denominator ratio back
across partitions with stride-0 SBUF DMAs.
"""

import math
from contextlib import ExitStack

import numpy as np
import ml_dtypes

B, S, D = 2, 1024, 1024
H, HD = 16, 64
CB, SB, J, W = 16, 8, 2, 256
Nc = S // CB  # 64 compressed blocks
T = B * S  # 2048 tokens total
NCORES = 8
HPC = H // NCORES  # 2 heads per core
C = HPC * HD  # 128 feature cols per core
INV = 1.0 / math.sqrt(HD)

bf16 = ml_dtypes.bfloat16

_PROG = None  # cached nc


def _build_program():
    import concourse.bass as bass
    import concourse.bacc as bacc
    import concourse.mybir as mybir
    import concourse.tile as tile
    from concourse.masks import make_identity

    dt = mybir.dt
    Alu = mybir.AluOpType
    Act = mybir.ActivationFunctionType

    nc = bacc.Bacc("TRN2", target_bir_lowering=False, debug=False,
                   num_devices=NCORES)

    # ---- DRAM parameters (per-core data supplied by the host) ----
    xT32 = nc.declare_dram_parameter("xT32", [D, T], dt.float32, isOutput=False)
    xT16 = nc.declare_dram_parameter("xT16", [D, T], dt.bfloat16, isOutput=False)
    wkwq = nc.declare_dram_parameter("wkwq", [D, D], dt.float32, isOutput=False)
    wq16 = nc.declare_dram_parameter("wq16", [D, C], dt.bfloat16, isOutput=False)
    wk16 = nc.declare_dram_parameter("wk16", [D, C], dt.bfloat16, isOutput=False)
    wv16 = nc.declare_dram_parameter("wv16", [D, C], dt.bfloat16, isOutput=False)
    wo16 = nc.declare_dram_parameter("wo16", [C, D], dt.bfloat16, isOutput=False)
    wg16 = nc.declare_dram_parameter("wg16", [D, 3], dt.bfloat16, isOutput=False)
    wpeT = nc.declare_dram_parameter("wpeT", [C, CB], dt.float32, isOutput=False)
    wkcB = nc.declare_dram_parameter("wkcB", [128, CB], dt.float32, isOutput=False)
    wvcB = nc.declare_dram_parameter("wvcB", [128, CB], dt.float32, isOutput=False)
    bgcol = nc.declare_dram_parameter("bgcol", [3, 1], dt.float32, isOutput=False)
    iota64 = nc.declare_dram_parameter("iota64", [128, Nc], dt.float32, isOutput=False)
    mask0 = nc.declare_dram_parameter("mask0", [128, 128], dt.float32, isOutput=False)
    mask2 = nc.declare_dram_parameter("mask2", [128, 128], dt.float32, isOutput=False)

    out_part = nc.declare_dram_parameter("out_part", [T, D], dt.float32, isOutput=True)

    with tile.TileContext(nc) as tc, ExitStack() as ctx:
        sync = nc.sync

        def pool(name, bufs=1, space="SBUF"):
            return ctx.enter_context(tc.tile_pool(name=name, bufs=bufs, space=space))

        def arr(p, n, shape, dtype, tag):
            return [p.tile(shape, dtype, tag=f"{tag}{i}", name=f"{tag}{i}")
                    for i in range(n)]

        # pools for phase A only (x fp32/bf16, WkWq^T, M, top-k scratch);
        # released after phase 3 so branch-phase pools can reuse the SBUF
        ctxA = ExitStack()

        def poolA(name, bufs=1):
            return ctxA.enter_context(tc.tile_pool(name=name, bufs=bufs))

        # ---------- constants ----------
        cpool = pool("consts")
        c_wkc = cpool.tile([128, CB], dt.float32, tag="wkc", name="wkc")
        c_wvc = cpool.tile([128, CB], dt.float32, tag="wvc", name="wvc")
        c_wpeT = cpool.tile([C, CB], dt.float32, tag="wpeT", name="wpeT")
        c_bg = cpool.tile([3, 1], dt.float32, tag="bg", name="bg")
        c_iota = cpool.tile([128, Nc], dt.float32, tag="iota", name="iota")
        c_m0 = cpool.tile([128, 128], dt.float32, tag="m0", name="m0")
        c_m2 = cpool.tile([128, 128], dt.float32, tag="m2", name="m2")
        c_ident = cpool.tile([128, 128], dt.bfloat16, tag="ident", name="ident")
        c_id32 = cpool.tile([128, 128], dt.float32, tag="id32", name="id32")
        for t_, src in [
            (c_wkc, wkcB), (c_wvc, wvcB), (c_wpeT, wpeT), (c_bg, bgcol),
            (c_iota, iota64), (c_m0, mask0), (c_m2, mask2),
        ]:
            sync.dma_start(t_[:], src[:])
        make_identity(nc, c_ident[:])
        make_identity(nc, c_id32[:])

        wpool = pool("wpool")
        wq = arr(wpool, 8, [128, C], dt.bfloat16, "wq_")
        wk = arr(wpool, 8, [128, C], dt.bfloat16, "wk_")
        wv = arr(wpool, 8, [128, C], dt.bfloat16, "wv_")
        wg = arr(wpool, 8, [128, 3], dt.bfloat16, "wg_")
        for i in range(8):
            sl = slice(i * 128, (i + 1) * 128)
            sync.dma_start(wq[i][:], wq16[sl, :])
            sync.dma_start(wk[i][:], wk16[sl, :])
            sync.dma_start(wv[i][:], wv16[sl, :])
            sync.dma_start(wg[i][:], wg16[sl, :])

        # PSUM pools: psA (2 banks) + psS (4 banks) + psb2 (2 banks) = 8
        psA = pool("psA", bufs=2, space="PSUM")
        psS = pool("psS", bufs=4, space="PSUM")
        psb2 = pool("psb2", bufs=1, space="PSUM")

        def pa():
            return psA.tile([128, 512], dt.float32, tag="pa", name="pa")

        def ps(p_=128, f=512):
            t_ = psS.tile([128, 512], dt.float32, tag="ps", name="ps")
            return t_[0:p_, 0:f]

        def psT(p_=128, f=512):
            t_ = psS.tile([128, 512], dt.bfloat16, tag="ps", name="psT")
            return t_[0:p_, 0:f]

        # ---------- persistent pools (created before phase-A scope: LIFO) ----------
        cxp = pool("cxp")
        cxkT = arr(cxp, 8, [128, 128], dt.float32, "cxk_")
        actp = pool("actp")
        qT = actp.tile([C, T], dt.bfloat16, tag="qT", name="qT")
        kT = actp.tile([C, T], dt.bfloat16, tag="kT", name="kT")
        vT = actp.tile([C, T], dt.bfloat16, tag="vT", name="vT")
        growb = actp.tile([3, T], dt.bfloat16, tag="growb", name="growb")
        otp = pool("otp")
        # OTS[rank]: [128 = (src n), 1024 s] bf16; masks output-batch `rank`
        OTS = [otp.tile([128, S], dt.bfloat16, tag=f"OTS{r}", name=f"OTS{r}")
               for r in range(2)]

        # ---------- big inputs (phase-A scoped pools) ----------
        xt32 = arr(poolA("xt32p"), 8, [128, T], dt.float32, "xt32_")
        wkwqt = arr(poolA("wkwqp"), 8, [128, D], dt.float32, "wkwq_")
        xt16 = arr(poolA("xt16p"), 8, [128, T], dt.bfloat16, "xt16_")
        for i in range(8):
            sl = slice(i * 128, (i + 1) * 128)
            sync.dma_start(xt32[i][:], xT32[sl, :])
            sync.dma_start(wkwqt[i][:], wkwq[sl, :])
            sync.dma_start(xt16[i][:], xT16[sl, :])

        # =========================================================
        # Phase 1: cxkT [d,128bn] f32 via mult + segmented reduce
        # cxk[d, b*64+n] = sum_t wkc[t] * xT[d, b*1024 + n*16 + t]
        # =========================================================
        cxyp = poolA("cxyp", bufs=2)
        wkc_b = c_wkc[:][:, None, :].to_broadcast((128, 128, CB))
        for i in range(8):
            y = cxyp.tile([128, T], dt.float32, tag="cxy", name="cxy")
            yv = y[:].rearrange("p (bn t) -> p bn t", t=CB)
            nc.vector.tensor_tensor(
                out=yv, in0=xt32[i][:].rearrange("p (bn t) -> p bn t", t=CB),
                in1=wkc_b, op=Alu.mult)
            nc.vector.tensor_reduce(out=cxkT[i][:], in_=yv,
                                    axis=mybir.AxisListType.X, op=Alu.add)

        # =========================================================
        # Phase 2: projections qT/kT/vT [C, T] bf16 (+pe for k,v), gates
        # =========================================================
        for ch in range(4):
            csl = slice(ch * 512, (ch + 1) * 512)
            for which, wtile, dest in (("q", wq, qT), ("k", wk, kT), ("v", wv, vT)):
                pp = pa()
                for i in range(8):
                    nc.tensor.matmul(pp[:], wtile[i][:], xt16[i][:, csl],
                                     start=(i == 0), stop=(i == 7))
                if which == "q":
                    nc.scalar.copy(dest[:, csl], pp[:])
                else:
                    # add tiled positional rows: out = psum + wpeT (tok%16)
                    dv = dest[:, csl].rearrange("p (r t) -> p r t", t=CB)
                    pv = pp[:].rearrange("p (r t) -> p r t", t=CB)
                    peb = c_wpeT[:][:, None, :].to_broadcast((C, 512 // CB, CB))
                    nc.vector.tensor_tensor(out=dv, in0=pv, in1=peb, op=Alu.add)
            pg = ps(3, 512)
            for i in range(8):
                nc.tensor.matmul(pg[:], wg[i][:], xt16[i][:, csl],
                                 start=(i == 0), stop=(i == 7))
            nc.scalar.activation(growb[:, csl], pg[:], Act.Sigmoid, bias=c_bg[:])

        # =========================================================
        # Phase 3: M[e,128bn] f32 = sum_d wkwq[d,e]*cxkT[d,bn]; imp; top2; OTS
        # =========================================================
        Mt = arr(poolA("mp"), 8, [128, 128], dt.float32, "M_")
        for e in range(8):
            pm = ps(128, 128)
            for i in range(8):
                nc.tensor.matmul(pm[:], wkwqt[i][:, e * 128:(e + 1) * 128],
                                 cxkT[i][:], start=(i == 0), stop=(i == 7))
            nc.scalar.copy(Mt[e][:], pm[:])

        idxp = poolA("idxp", bufs=3)
        for b in range(B):
            for st in range(8):
                ssl = slice(b * S + st * 128, b * S + (st + 1) * 128)
                pi = ps(128, Nc)
                for e in range(8):
                    nc.tensor.matmul(pi[:], xt32[e][:, ssl],
                                     Mt[e][:, b * Nc:(b + 1) * Nc],
                                     start=(e == 0), stop=(e == 7))
                impt = idxp.tile([128, Nc], dt.float32, tag="impt", name="impt")
                nc.vector.tensor_copy(impt[:], pi[:])
                mx8 = idxp.tile([128, 8], dt.float32, tag="mx8", name="mx8")
                ix8 = idxp.tile([128, 8], dt.uint32, tag="ix8", name="ix8")
                nc.vector.max(out=mx8[:], in_=impt[:])
                nc.vector.max_index(out=ix8[:], in_max=mx8[:], in_values=impt[:])
                ixf = idxp.tile([128, 2], dt.float32, tag="ixf", name="ixf")
                nc.vector.tensor_copy(ixf[:], ix8[:, 0:2])
                for r in range(2):
                    oh = idxp.tile([128, Nc], dt.bfloat16, tag="oh", name="oh")
                    nc.vector.tensor_scalar(
                        out=oh[:], in0=c_iota[:], scalar1=ixf[:, r:r + 1],
                        scalar2=None, op0=Alu.is_equal)
                    # transpose into the src-half psum partitions so the copy
                    # into the stacked OTS tile keeps partition bases equal
                    ptf = psS.tile([128, 512], dt.bfloat16, tag="ps", name="psT")
                    pt = ptf[b * Nc:(b + 1) * Nc, 0:128]
                    nc.tensor.transpose(pt, oh[:], c_ident[:])
                    nc.vector.tensor_copy(
                        OTS[r][b * Nc:(b + 1) * Nc, st * 128:(st + 1) * 128],
                        pt)

        ctxA.close()  # release xT32/wkwq/xT16/M/idx SBUF for later phases

        latep = pool("latep")
        wo2 = latep.tile([C, D], dt.bfloat16, tag="wo2", name="wo2")
        sync.dma_start(wo2[:], wo16[:])

        # =========================================================
        # Phase 4: compress kT/vT -> ckTs [C,128bn], cva; v_tok; Vblk2; kSel
        # (kT/vT already include the tiled pe rows, so no c1k/c1v terms)
        # =========================================================
        ckp = pool("ckp")
        ckTs = ckp.tile([C, 128], dt.bfloat16, tag="ckTs", name="ckTs")
        ck32 = ckp.tile([C, 128], dt.float32, tag="ck32", name="ck32")
        cvT = ckp.tile([C, 128], dt.float32, tag="cvT", name="cvT")
        # cva[b]: [n, h*65+c] bf16 with ones col at 64/129 (per batch)
        cva = [ckp.tile([Nc, 130], dt.bfloat16, tag=f"cva{b}", name=f"cva{b}")
               for b in range(B)]
        cmpp = pool("cmpp", bufs=2)
        for src_t, wt, tgt in ((kT, c_wkc, ck32), (vT, c_wvc, cvT)):
            y = cmpp.tile([C, T], dt.bfloat16, tag="cy", name="cy")
            yv = y[:].rearrange("p (bn t) -> p bn t", t=CB)
            nc.vector.tensor_tensor(
                out=yv, in0=src_t[:].rearrange("p (bn t) -> p bn t", t=CB),
                in1=wt[:][:, None, :].to_broadcast((128, 128, CB)), op=Alu.mult)
            nc.vector.tensor_reduce(out=tgt[:], in_=yv,
                                    axis=mybir.AxisListType.X, op=Alu.add)
        nc.vector.tensor_copy(ckTs[:], ck32[:])
        for b in range(B):
            pv = ps(Nc, 128)
            nc.tensor.transpose(pv[:], cvT[:, b * Nc:(b + 1) * Nc], c_id32[:])
            dv = cva[b][:].rearrange("p (h c) -> p h c", h=2)
            nc.vector.tensor_copy(
                dv[:, :, 0:64], pv[:].rearrange("p (h c) -> p h c", h=2))
            nc.gpsimd.memset(dv[:, :, 64:65], 1.0)

        # v_tok: [tok, 130] bf16 x16 tiles (natural order, for branch 3)
        v_tok = arr(pool("vtp"), 16, [128, 130], dt.bfloat16, "vtok_")
        for tt in range(16):
            pv = psT(128, 128)
            nc.tensor.transpose(pv[:], vT[:, tt * 128:(tt + 1) * 128], c_ident[:])
            dv = v_tok[tt][:].rearrange("p (h c) -> p h c", h=2)
            nc.vector.tensor_copy(
                dv[:, :, 0:64], pv[:].rearrange("p (h c) -> p h c", h=2))
            nc.gpsimd.memset(dv[:, :, 64:65], 1.0)

        # Vblk2[t]: [128 = (src n), 130] bf16 — selectable tokens src*1024+n*16+t
        # kSel[t]:  [128 c, 128 = (src n)] bf16 — their keys, feature-major
        vbp = pool("vbp")
        Vblk2 = [vbp.tile([128, 130], dt.bfloat16, tag=f"vb{t}", name=f"vb{t}")
                 for t in range(SB)]
        kSel = [vbp.tile([128, 128], dt.bfloat16, tag=f"ks{t}", name=f"ks{t}")
                for t in range(SB)]
        vTv = vT[:].rearrange("p (b n t) -> p b t n", b=B, t=CB)
        kTv = kT[:].rearrange("p (b n t) -> p b t n", b=B, t=CB)
        for t in range(SB):
            for src in range(2):
                pvf = psS.tile([128, 512], dt.bfloat16, tag="ps", name="psT")
                pv = pvf[src * Nc:(src + 1) * Nc, 0:128]
                nc.tensor.transpose(pv, vTv[:, src, t, :], c_ident[:])
                dv = Vblk2[t][src * Nc:(src + 1) * Nc, :].rearrange(
                    "p (h c) -> p h c", h=2)
                nc.vector.tensor_copy(
                    dv[:, :, 0:64], pv.rearrange("p (h c) -> p h c", h=2))
                nc.gpsimd.memset(dv[:, :, 64:65], 1.0)
                nc.vector.tensor_copy(
                    kSel[t][:, src * Nc:(src + 1) * Nc], kTv[:, src, t, :])

        # =========================================================
        # Branches. ofull[k][b][h]: [65, S] bf16
        #   rows 0-63 = numerator values, row 64 = softmax denominator.
        # =========================================================
        brp = pool("brp")
        ofull = [[[brp.tile([HD + 1, S], dt.bfloat16, tag=f"o{k}_{b}{h}",
                            name=f"o{k}_{b}{h}")
                   for h in range(2)] for b in range(2)] for k in range(3)]
        expp = pool("expp", bufs=4)

        # ---- branch 1: compressed attention ----
        for b in range(B):
            for h in range(HPC):
                hsl = slice(h * HD, (h + 1) * HD)
                asl = slice(h * 65, (h + 1) * 65)
                p1 = expp.tile([Nc, S], dt.bfloat16, tag="p1t", name="p1t")
                for ch in range(2):
                    csl = slice(ch * 512, (ch + 1) * 512)
                    tsl = slice(b * S + ch * 512, b * S + (ch + 1) * 512)
                    pp = ps(Nc, 512)
                    nc.tensor.matmul(pp[:], ckTs[hsl, b * Nc:(b + 1) * Nc],
                                     qT[hsl, tsl], start=True, stop=True)
                    nc.scalar.activation(p1[:, csl], pp[:], Act.Exp, scale=INV)
                for ch in range(2):
                    csl = slice(ch * 512, (ch + 1) * 512)
                    po = ps(HD + 1, 512)
                    nc.tensor.matmul(po[:], cva[b][:, asl],
                                     p1[:, csl], start=True, stop=True)
                    nc.scalar.copy(ofull[0][b][h][:, csl], po[:])

        # ---- branch 3: causal sliding window (strip-tiled scores) ----
        strp = pool("strp", bufs=2)
        for b in range(B):
            for h in range(HPC):
                hsl = slice(h * HD, (h + 1) * HD)
                asl = slice(h * 65, (h + 1) * 65)
                p3s = [strp.tile([128, 512], dt.bfloat16, tag=f"p3s{kt}",
                                 name=f"p3s{kt}") for kt in range(8)]
                for kt in range(8):
                    nst = min(3, 8 - kt)
                    strip = nst * 128
                    ktsl = slice(b * S + kt * 128, b * S + (kt + 1) * 128)
                    qsl = slice(b * S + kt * 128, b * S + kt * 128 + strip)
                    pp = ps(128, 512)
                    nc.tensor.matmul(pp[:, 0:strip], kT[hsl, ktsl],
                                     qT[hsl, qsl], start=True, stop=True)
                    # diag (st==kt): causal mask; st==kt+2: window mask
                    nc.vector.tensor_tensor(out=pp[:, 0:128], in0=pp[:, 0:128],
                                            in1=c_m0[:], op=Alu.add)
                    if nst == 3:
                        nc.vector.tensor_tensor(out=pp[:, 256:384],
                                                in0=pp[:, 256:384],
                                                in1=c_m2[:], op=Alu.add)
                    nc.scalar.activation(p3s[kt][:, 0:strip], pp[:, 0:strip],
                                         Act.Exp, scale=INV)
                for sg in range(2):  # groups of 4 query-chunks
                    po4 = ps(HD + 1, 512)
                    for si in range(4):
                        st = sg * 4 + si
                        kts = [kt for kt in (st - 2, st - 1, st) if kt >= 0]
                        for ki, kt in enumerate(kts):
                            nc.tensor.matmul(
                                po4[:, si * 128:(si + 1) * 128],
                                v_tok[b * 8 + kt][:, asl],
                                p3s[kt][:, (st - kt) * 128:(st - kt + 1) * 128],
                                start=(ki == 0), stop=(ki == len(kts) - 1),
                                skip_group_check=True)
                    nc.scalar.copy(ofull[2][b][h][:, sg * 512:(sg + 1) * 512],
                                   po4[:])

        # ---- branch 2: selected-block attention (dense + one-hot mask) ----
        for b in range(B):
            for h in range(HPC):
                hsl = slice(h * HD, (h + 1) * HD)
                asl = slice(h * 65, (h + 1) * 65)
                po2 = [psb2.tile([HD + 1, 512], dt.float32, tag=f"acc{ch}",
                                 name=f"acc{ch}") for ch in range(2)]
                for t in range(SB):
                    p2 = expp.tile([128, S], dt.bfloat16, tag="p2t", name="p2t")
                    for ch in range(2):
                        csl = slice(ch * 512, (ch + 1) * 512)
                        tsl = slice(b * S + ch * 512, b * S + (ch + 1) * 512)
                        pp = ps(128, 512)
                        nc.tensor.matmul(pp[:], kSel[t][hsl, :],
                                         qT[hsl, tsl], start=True, stop=True)
                        nc.scalar.activation(p2[:, csl], pp[:], Act.Exp,
                                             scale=INV)
                    # mask with one-hot of the selected blocks (both srcs)
                    nc.vector.tensor_tensor(out=p2[:], in0=p2[:],
                                            in1=OTS[b][:], op=Alu.mult)
                    for ch in range(2):
                        csl = slice(ch * 512, (ch + 1) * 512)
                        nc.tensor.matmul(
                            po2[ch][:], Vblk2[t][:, asl], p2[:, csl],
                            start=(t == 0), stop=(t == SB - 1))
                for ch in range(2):
                    csl = slice(ch * 512, (ch + 1) * 512)
                    nc.scalar.copy(ofull[1][b][h][:, csl], po2[ch][:])

        # =========================================================
        # Phase 8: gating. alpha_k(s) = g_k(s)/den_k(s) computed with s on
        # partitions (transpose den+gates, reciprocal, transpose back), then
        # broadcast across 64 hd-partitions with stride-0 SBUF DMAs.
        # gatedT [128 = (h hd), T] bf16 = sum_k alpha_k * o_k
        # =========================================================
        gp = pool("gp", bufs=2)
        dg9 = [gp.tile([9, S], dt.bfloat16, tag=f"dg9{b}", name=f"dg9{b}")
               for b in range(B)]
        a6 = [gp.tile([6, S], dt.bfloat16, tag=f"a6{b}", name=f"a6{b}")
              for b in range(B)]
        for b in range(B):
            nc.vector.tensor_copy(dg9[b][0:3, :], growb[:, b * S:(b + 1) * S])
            for k in range(3):
                for h in range(HPC):
                    sync.dma_start(dg9[b][3 + k * 2 + h:4 + k * 2 + h, :],
                                   ofull[k][b][h][HD:HD + 1, :])
        rcp = pool("rcp", bufs=4)
        for b in range(B):
            for st in range(8):
                ssl = slice(st * 128, (st + 1) * 128)
                pt = psT(128, 9)
                nc.tensor.transpose(pt[:], dg9[b][:, ssl], c_ident[0:9, 0:9])
                rc = rcp.tile([128, 6], dt.float32, tag="rc", name="rc")
                nc.vector.reciprocal(rc[:], pt[:, 3:9])
                at = rcp.tile([128, 6], dt.bfloat16, tag="at", name="at")
                nc.vector.tensor_tensor(
                    out=at[:].rearrange("p (k h) -> p k h", k=3),
                    in0=rc[:].rearrange("p (k h) -> p k h", k=3),
                    in1=pt[:, 0:3][:, :, None].to_broadcast((128, 3, 2)),
                    op=Alu.mult)
                pb = psT(6, 128)
                nc.tensor.transpose(pb[:], at[:], c_ident[:])
                nc.vector.tensor_copy(a6[b][:, ssl], pb[:])

        gatedT = gp.tile([C, T], dt.bfloat16, tag="gatedT", name="gatedT")
        c_ones1 = gp.tile([1, HD], dt.bfloat16, tag="ones1", name="ones1")
        nc.gpsimd.memset(c_ones1[:], 1.0)
        # alpha rows as separate [1, S] tiles so matmul base partition is 0
        ar = [[gp.tile([1, S], dt.bfloat16, tag=f"ar{b}{r}", name=f"ar{b}{r}")
               for r in range(6)] for b in range(B)]
        for b in range(B):
            for r in range(6):
                sync.dma_start(ar[b][r][:], a6[b][r:r + 1, :])
        rgp = pool("rgp", bufs=3)
        for b in range(B):
            for h in range(HPC):
                for ch in range(2):
                    csl = slice(ch * 512, (ch + 1) * 512)
                    tsl = slice(b * S + ch * 512, b * S + (ch + 1) * 512)
                    # accumulate at base partition 0; h=1 is DMA-shifted into
                    # the stacked gatedT afterwards (engines keep partitions)
                    acc = rgp.tile([HD, 512], dt.bfloat16, tag="gacc",
                                   name="gacc") if h == 1 else None
                    dst = acc[:] if h == 1 else gatedT[0:HD, tsl]
                    for k in range(3):
                        row = k * 2 + h
                        # broadcast alpha row across 64 hd-partitions on the PE
                        pr = ps(HD, 512)
                        nc.tensor.matmul(pr[:], c_ones1[:],
                                         ar[b][row][:, csl],
                                         start=True, stop=True)
                        ob = ofull[k][b][h][0:HD, csl]
                        if k == 0:
                            nc.vector.tensor_tensor(out=dst, in0=ob, in1=pr[:],
                                                    op=Alu.mult)
                        else:
                            tmp = rgp.tile([HD, 512], dt.bfloat16, tag="gtmp",
                                           name="gtmp")
                            nc.vector.tensor_tensor(out=tmp[:], in0=ob,
                                                    in1=pr[:], op=Alu.mult)
                            nc.vector.tensor_tensor(out=dst, in0=dst,
                                                    in1=tmp[:], op=Alu.add)
                    if h == 1:
                        sync.dma_start(gatedT[HD:C, tsl], acc[:])

        # =========================================================
        # Phase 9: out_part[s,:] = gatedT^T @ wo2 (both heads, c=128; bias on host)
        # =========================================================
        fop = pool("fop", bufs=3)
        for st in range(16):
            ssl = slice(st * 128, (st + 1) * 128)
            ot = fop.tile([128, D], dt.float32, tag="fo", name="fo")
            for ch in range(2):
                csl = slice(ch * 512, (ch + 1) * 512)
                pf = pa()
                nc.tensor.matmul(pf[:], gatedT[:, ssl], wo2[:, csl],
                                 start=True, stop=True)
                nc.vector.tensor_copy(ot[:, csl], pf[:])
            sync.dma_start(out_part[ssl, :], ot[:])

    nc.compile()
    return nc


def _prep_inputs(inputs):
    """Build the 8 per-core input maps from the full problem inputs."""
    x = np.ascontiguousarray(np.asarray(inputs["x"], dtype=np.float32))
    Wq = np.asarray(inputs["Wq"], dtype=np.float32)
    Wk = np.asarray(inputs["Wk"], dtype=np.float32)
    Wv = np.asarray(inputs["Wv"], dtype=np.float32)
    Wo = np.asarray(inputs["Wo"], dtype=np.float32)
    Wg = np.asarray(inputs["Wg"], dtype=np.float32)
    bg = np.asarray(inputs["bg"], dtype=np.float32)
    wkc = np.asarray(inputs["wkc"], dtype=np.float32)
    wvc = np.asarray(inputs["wvc"], dtype=np.float32)
    wpe = np.asarray(inputs["wpe"], dtype=np.float32)

    xT = np.ascontiguousarray(x.reshape(T, D).T)          # [D, T] f32
    xT16 = xT.astype(bf16)
    wkwq = np.ascontiguousarray(Wk @ Wq.T)                # [D, D] f32
    iota = np.tile(np.arange(Nc, dtype=np.float32), (128, 1))
    ii = np.arange(128)[:, None]
    jj = np.arange(128)[None, :]
    m0 = np.where(jj >= ii, 0.0, -1e6).astype(np.float32)  # delta=0 keep col>=p
    m2 = np.where(jj <= ii, 0.0, -1e6).astype(np.float32)  # delta=2 keep col<=p
    wkcB = np.ascontiguousarray(np.tile(wkc, (128, 1)).astype(np.float32))
    wvcB = np.ascontiguousarray(np.tile(wvc, (128, 1)).astype(np.float32))
    bgcol = np.ascontiguousarray(bg.reshape(3, 1).astype(np.float32))

    in_maps = []
    for i in range(NCORES):
        csl = slice(i * C, (i + 1) * C)
        m = {
            "xT32": xT,
            "xT16": xT16,
            "wkwq": wkwq,
            "wq16": np.ascontiguousarray(Wq[:, csl]).astype(bf16),
            "wk16": np.ascontiguousarray(Wk[:, csl]).astype(bf16),
            "wv16": np.ascontiguousarray(Wv[:, csl]).astype(bf16),
            "wo16": np.ascontiguousarray(Wo[csl, :]).astype(bf16),
            "wg16": np.ascontiguousarray(Wg).astype(bf16),
            "wpeT": np.ascontiguousarray(wpe.T[csl, :]).astype(np.float32),
            "wkcB": wkcB,
            "wvcB": wvcB,
            "bgcol": bgcol,
            "iota64": iota,
            "mask0": m0,
            "mask2": m2,
        }
        in_maps.append(m)
    return in_maps


_LAST_RESULTS = None


def kernel(**inputs) -> np.ndarray:
    global _PROG, _LAST_RESULTS
    import os
    from concourse.bass_utils import run_bass_kernel_spmd

    if _PROG is None:
        _PROG = _build_program()
    nc = _PROG

    in_maps = _prep_inputs(inputs)
    trace = bool(int(os.environ.get("KERNEL_TRACE", "0")))
    res = run_bass_kernel_spmd(nc, in_maps, core_ids=list(range(NCORES)),
                               trace=trace)
    _LAST_RESULTS = res
    total = np.zeros((T, D), np.float32)
    for i in range(NCORES):
        total += res.results[i]["out_part"]
    total += np.asarray(inputs["bo"], dtype=np.float32)[None, :]
    return total.reshape(B, S, D)
